# revision 2
# baseline (speedup 1.0000x reference)
"""LocallyConnected1d Trainium2 kernel (v9: HBM-traffic-optimized).

out[b, o, l] = sum_{c,k} x[b, c, l+k] * weight[o, c, l, k] + bias[o, l]
  x: (32, 128, 2050) f32, weight: (128, 128, 2048, 3) f32, bias: (128, 2048) f32
  out: (32, 128, 2048) f32

Sharding: sequence-parallel over L across 8 cores; each core owns 256 output
positions, its private weight slice, a 258-wide x window, and a bias slice.

The kernel is HBM-bound: the v8 trace shows the DMA stream at ~340 GB/s
(89% of the 358 GB/s per-core HBM roof) for the whole run while the PE
(41 us floor) waits on weight tiles.  v9 therefore minimizes bytes and
queue serialization:

  weights  12.58 MB fp8-e3m4 (read once; global 2^-5 scale folded into x)
  x         2.11 MB bf16, host pre-transposed to (c, w, b)
  bias      1.05 MB fp8 of (16*bias), host-replicated over the 32 batch
            partitions; expanded to bf16 in SBUF by a casting SWDGE DMA
  out       2.10 MB bf16 of 16*(w.x+b); host divides by 16 (exact pow2)

The 16x bias scale keeps fp8(16b) in e3m4's normal range: x is pre-scaled
by 2^-1 instead of 2^-5, so PSUM holds 16*(w.x) and the DVE drain adds the
fp8 bias image directly.  All scales are powers of two -- exact in bf16.

Issue paths (three independent engine FIFOs feeding the 16 SDMA engines):
  SP   (sync HWDGE): weight tiles only.  Window 0 is split into 4 bank-
       sized pieces so the first matmul starts ~2 us earlier; window 15
       into halves so its tail overlaps the drain.
  ACT  (scalar HWDGE): the 4 x chunks up front, then one out tile per
       window (last window flushed per-bank).
  Pool (gpsimd SWDGE): 16 per-window bias DMAs, emitted up front, casting
       fp8 -> bf16 inline.  A separate descriptor ring, so bias can never
       queue behind x or outs.

Per-core compute (unchanged from v8): out.T[b, l, o] = sum_c x[c, m, b] *
W[o, c, l, m-l] per x column m.  The x column is the PE stationary operand
(K=128 c, M=32 b); the weights are the moving operand, host-packed in
exact consumption order (contiguous (l', o) blocks per (bank, x column))
because fp8 moving operands only stream at full rate when the innermost
dim is stride-1.  One PSUM bank takes 6 weight matmuls (x columns
m = 4j..4j+5 clipped to the bank); the first carries start=True.  The DVE
drains each bank with scalar_tensor_tensor (st = ps*1 + bias) fused into
the fp32->bf16 staging copy.
"""

import numpy as np
import ml_dtypes

BF16 = ml_dtypes.bfloat16
F8NP = ml_dtypes.float8_e3m4

import concourse.bass as bass
import concourse.mybir as mybir
import concourse.tile as tile
from concourse.vector_clock import ScopedClock, VectorClock
from concourse.bass_utils import run_bass_kernel_spmd

# ---------------------------------------------------------------------------
# Environment patches
# ---------------------------------------------------------------------------

# The walrus build in this image rejects instructions with >1 sem wait; the
# Tile tail drain carries one wait per logical processor.  Split them into
# single-wait nops on SP before the drain.
def _patched_drain_and_barrier(self, tick_clock, wait_clock):
    gc = tick_clock.global_clock
    n = len(gc)
    for proc in range(n):
        t = gc[proc]
        if t <= 0:
            continue
        single = VectorClock([0] * n)
        single.require_at_least(proc, t)
        inst = self.nc.sync.nop(hint="tail_drain_wait")
        wait_clock.add_sem_waits(inst.ins, ScopedClock({None: single}))
    self.nc.sync.drain()
    self.nc.all_engine_barrier()
    assert self.sems is not None
    popped = self.nc._tile_sem_poison_stack.pop()
    assert popped is self._sem_poison
    # Clear sems WITHOUT the trailing all-engine barrier: the clear runs on
    # one engine after the barrier above, and nothing after it reads sems.
    self.nc.clear_and_free_semaphores(list(self.sems.allocated().values()))


if not getattr(tile.TileContext, "_drain_patch_applied", False):
    tile.TileContext._drain_and_barrier = _patched_drain_and_barrier
    tile.TileContext._drain_patch_applied = True


def _split_multi_waits(nc: bass.Bass) -> int:
    """Hoist all but the last wait of any multi-wait instruction onto
    single-wait nops inserted just before it in its engine's program order
    (the hardware takes one sem wait per instruction; this walrus build
    rejects multi-wait instructions instead of splitting them)."""
    n_split = 0
    for f in nc.m.functions:
        for bb in f.blocks:
            insts = list(bb.instructions)
            out = []
            for inst in insts:
                si = inst.sync_info
                if si is not None and len(si.on_wait) > 1:
                    waits = list(si.on_wait)
                    for w in waits[:-1]:
                        nop = mybir.InstNoOp(
                            name=nc.get_next_instruction_name(),
                            engine=inst.engine,
                            ins=[],
                            outs=[],
                            sync_info=mybir.SyncInfo(on_wait=[w], on_update=[]),
                        )
                        out.append(nop)
                    si.on_wait = [waits[-1]]
                    n_split += 1
                out.append(inst)
            bb.instructions = out
    return n_split

# ---------------------------------------------------------------------------
# Problem constants (hardcoded from the module spec)
# ---------------------------------------------------------------------------
N_CORES = 8
B = 32
CIN = 128
COUT = 128
L = 2048
KS = 3
W_FULL = 2050

LSH = L // N_CORES          # 256 output positions per core
WW = LSH + KS - 1           # 258-wide x window per core

LT = 16                     # l positions per weight tile / staging window
NWIN = LSH // LT            # 16 windows per core
BANKL = 4                   # l positions per PSUM bank (4*128 = 512 fp32)
NBANK = LT // BANKL         # 4 banks per window
WFREE = COUT * LT * KS      # weight tile free size (6144 fp8 = 6 KB/part)
BANKF = COUT * BANKL * KS   # weight bytes per bank per partition (1536)

F32 = mybir.dt.float32
F16 = mybir.dt.bfloat16
F8 = mybir.dt.float8e3         # e3m4: 4 mantissa bits, max 15.5
WSCALE = 2.0 ** -5             # weight pre-scale so w/WSCALE fits e3m4
XSCALE = 2.0 ** -1             # x pre-scale: PSUM = (w/WSCALE)*(x*XSCALE)
                               #            = 16*w*x; bias ships as fp8(16*b)
OSCALE = WSCALE / XSCALE       # host multiplies gathered out by this (2^-4)

# per-bank x-column blocks: d = mw - lw0 in 0..5, with nl(d) l' rows each
# (nl = 1,2,3,3,2,1); DOFF[d] = block offset within the bank, in COUT units
DOFF = [0, 1, 3, 6, 9, 11]

# x chunk boundaries on the ACT queue: window lc needs cols < 16*lc + 18
XCHUNKS = [(0, LT + 2), (LT + 2, 5 * LT + 2), (5 * LT + 2, 9 * LT + 2),
           (9 * LT + 2, WW)]


def _weight_perm() -> np.ndarray:
    """Flat destination position (within a window's 6144-element image) for
    each source element ordered (l', k, o)."""
    pos = np.empty((LT, KS, COUT), dtype=np.int64)
    o = np.arange(COUT)
    for lp in range(LT):
        jb = lp // BANKL
        for k in range(KS):
            mw = lp + k
            d = mw - jb * BANKL
            lo = max(jb * BANKL, mw - (KS - 1))
            base = jb * BANKF + DOFF[d] * COUT + (lp - lo) * COUT
            pos[lp, k] = base + o
    return pos.reshape(-1)


_WPERM = _weight_perm()


def _build_nc(split: bool = True) -> bass.Bass:
    nc = bass.Bass()

    x_d = nc.declare_dram_parameter("xT", [CIN, WW, B], F16, isOutput=False)
    wt_d = nc.declare_dram_parameter("wt", [NWIN, CIN, WFREE], F8,
                                     isOutput=False)
    # bias pre-replicated by the host across the 32 batch partitions (DVE
    # lanes cannot read across partitions) and pre-scaled by 16 so it ships
    # as fp8; the SWDGE DMA expands it to bf16 in SBUF.
    br_d = nc.declare_dram_parameter("biasR", [B, LSH, COUT], F8,
                                     isOutput=False)
    # (b, l, o) layout: staging DMAs out as contiguous runs; the host
    # transposes back after gather.
    out_d = nc.declare_dram_parameter("out", [B, LSH, COUT], F16, isOutput=True)

    with tile.TileContext(nc) as tc:
        with (
            tc.tile_pool(name="xp", bufs=1) as xp,
            tc.tile_pool(name="btp", bufs=1) as btp,
            tc.tile_pool(name="wp", bufs=8) as wp,
            tc.tile_pool(name="sp", bufs=6) as sp,
            tc.tile_pool(name="pp", bufs=8, space="PSUM") as pp,
        ):
            # Persistent x in (c, w, b) layout: the stationary operand for
            # column m is x_sb[:, m, :] (K=128 c, M=32 b).  Host pre-
            # transposed, so both DMA sides are fully contiguous.  The first
            # chunk covers only window 0 so the first matmul starts ASAP.
            x_sb = xp.tile([CIN, WW, B], F16)
            for a, b_ in XCHUNKS:
                nc.scalar.dma_start(x_sb[:, a:b_, :], x_d[:, a:b_, :])

            # All bias windows up front on the gpsimd SWDGE ring (its own
            # descriptor path -- never queues behind x or outs), fp8 in HBM
            # expanded to bf16 in SBUF by the DMA's inline cast.
            b_all = btp.tile([B, LSH, COUT], F16)
            for w in range(NWIN):
                nc.gpsimd.dma_start(b_all[:, w * LT:(w + 1) * LT, :],
                                    br_d[:, w * LT:(w + 1) * LT, :])

            for lc in range(NWIN):
                # weight tile, host-packed in matmul consumption order: for
                # each bank jb and x column d, a contiguous (l', o) block.
                # fp8 moving operands stream at full rate ONLY when the
                # innermost dim is stride-1.  One big DMA per window (small
                # DMAs tank the stream rate); window 0 is split per-bank so
                # bank 0's matmuls start ~2us earlier, window 15 in halves
                # so its compute overlaps the stream tail.
                w_t = wp.tile([CIN, WFREE], F8, tag="w", name="w_t")
                if lc == 0:
                    for jb in range(NBANK):
                        nc.sync.dma_start(
                            w_t[:, jb * BANKF:(jb + 1) * BANKF],
                            wt_d[lc, :, jb * BANKF:(jb + 1) * BANKF])
                elif lc == NWIN - 1:
                    half = WFREE // 2
                    nc.sync.dma_start(w_t[:, 0:half], wt_d[lc, :, 0:half])
                    nc.sync.dma_start(w_t[:, half:WFREE],
                                      wt_d[lc, :, half:WFREE])
                else:
                    nc.sync.dma_start(w_t[:], wt_d[lc])

                st = sp.tile([B, LT, COUT], F16, tag="st", name=f"st_{lc}")

                for jb in range(NBANK):
                    ps = pp.tile([B, BANKL, COUT], F32, tag="ps", name="ps")
                    lw0 = jb * BANKL              # window-local l of bank start

                    # six weight matmuls: x columns m = bank start .. +5;
                    # each reads one fully contiguous nl*COUT-element block.
                    # d=0 carries start=True: clears the bank's has_written
                    # bits, so each element's first matmul overwrites and
                    # later ones accumulate -- no separate init matmul.
                    for d in range(BANKL + KS - 1):
                        mw = lw0 + d                  # window-local x column
                        m = lc * LT + mw              # shard-local x column
                        lo = max(lw0, mw - (KS - 1))  # window-local l' range
                        hi = min(lw0 + BANKL - 1, mw)
                        nl = hi - lo + 1
                        rhs = bass.AP(
                            w_t[:].tensor,
                            jb * BANKF + DOFF[d] * COUT,
                            [[WFREE, CIN], [1, nl * COUT]],
                        )
                        nc.tensor.matmul(
                            ps[:, lo - lw0:hi - lw0 + 1, :],
                            x_sb[:, m, :],
                            rhs,
                            start=(d == 0),
                            stop=(d == BANKL + KS - 2),
                            skip_group_check=True,
                        )

                    # staging = PSUM + bias, fused into the DVE copy:
                    # st = (ps * 1.0) + bias  (fp32 PSUM -> bf16 SBUF)
                    nc.vector.scalar_tensor_tensor(
                        st[:, lw0:lw0 + BANKL, :],
                        ps[:],
                        1.0,
                        b_all[:, lc * LT + lw0:lc * LT + lw0 + BANKL, :],
                        op0=mybir.AluOpType.mult,
                        op1=mybir.AluOpType.add,
                    )

                if lc < NWIN - 1:
                    nc.scalar.dma_start(out_d[:, lc * LT:(lc + 1) * LT, :],
                                        st[:])
                else:
                    # last window: per-bank flushes so the kernel tail is one
                    # small transfer instead of a whole-window one
                    for jb in range(NBANK):
                        l0 = lc * LT + jb * BANKL
                        nc.scalar.dma_start(
                            out_d[:, l0:l0 + BANKL, :],
                            st[:, jb * BANKL:(jb + 1) * BANKL, :])

    if split:
        _split_multi_waits(nc)
    return nc


_NC_CACHE = None


def _get_nc() -> bass.Bass:
    global _NC_CACHE
    if _NC_CACHE is None:
        _NC_CACHE = _build_nc()
    return _NC_CACHE


def _tile_weights(w_shard: np.ndarray) -> np.ndarray:
    """(COUT, CIN, LSH, KS) -> (NWIN, CIN, WFREE) per-window SBUF tile
    images in matmul consumption order: contiguous (l', o) blocks per
    (bank, x-column), so every matmul rhs is one stride-1 run."""
    w = w_shard.transpose(1, 2, 3, 0)                  # (CIN, LSH, KS, COUT)
    w = w.reshape(CIN, NWIN, LT * KS * COUT)
    w = np.ascontiguousarray(w.transpose(1, 0, 2))     # (NWIN, CIN, LT*KS*COUT)
    out = np.empty_like(w)
    out[:, :, _WPERM] = w
    return out


def shard_inputs(x, weight, bias):
    x = (np.asarray(x, dtype=np.float32) * XSCALE).astype(BF16)
    weight = (np.asarray(weight, dtype=np.float32) * (1.0 / WSCALE)).astype(F8NP)
    bias = (np.asarray(bias, dtype=np.float32) * (1.0 / OSCALE)).astype(F8NP)
    xT = x.transpose(1, 2, 0)                          # (CIN, W_FULL, B)
    in_maps = []
    for i in range(N_CORES):
        l0 = i * LSH
        in_maps.append({
            "xT": np.ascontiguousarray(xT[:, l0:l0 + WW, :]),
            "wt": _tile_weights(weight[:, :, l0:l0 + LSH, :]),
            "biasR": np.ascontiguousarray(
                np.broadcast_to(bias[:, l0:l0 + LSH].T[None, :, :],
                                (B, LSH, COUT))),
        })
    return in_maps


def gather_output(results):
    out = np.empty((B, COUT, L), dtype=np.float32)
    for i in range(N_CORES):
        out[:, :, i * LSH:(i + 1) * LSH] = (
            results[i]["out"].astype(np.float32).transpose(0, 2, 1) * OSCALE)
    return out


def kernel(x, weight, bias):
    nc = _get_nc()
    in_maps = shard_inputs(x, weight, bias)
    res = run_bass_kernel_spmd(nc, in_maps, core_ids=list(range(N_CORES)),
                               trace=False)
    return gather_output(res.results)


# revision 6
# speedup vs baseline: 1.0988x; 1.0988x over previous
"""LocallyConnected1d Trainium2 kernel (v9: HBM-traffic-optimized).

out[b, o, l] = sum_{c,k} x[b, c, l+k] * weight[o, c, l, k] + bias[o, l]
  x: (32, 128, 2050) f32, weight: (128, 128, 2048, 3) f32, bias: (128, 2048) f32
  out: (32, 128, 2048) f32

Sharding: sequence-parallel over L across 8 cores; each core owns 256 output
positions, its private weight slice, a 258-wide x window, and a bias slice.

The kernel is HBM-bound: the v8 trace shows the DMA stream at ~340 GB/s
(89% of the 358 GB/s per-core HBM roof) for the whole run while the PE
(41 us floor) waits on weight tiles.  v9 therefore minimizes bytes and
queue serialization:

  weights  12.58 MB fp8-e3m4 (read once; global 2^-5 scale folded into x)
  x         2.11 MB bf16, host pre-transposed to (c, w, b)
  bias      1.05 MB fp8 of (16*bias), host-replicated over the 32 batch
            partitions; expanded to bf16 in SBUF by a casting SWDGE DMA
  out       2.10 MB bf16 of 16*(w.x+b); host divides by 16 (exact pow2)

The 16x bias scale keeps fp8(16b) in e3m4's normal range: x is pre-scaled
by 2^-1 instead of 2^-5, so PSUM holds 16*(w.x) and the DVE drain adds the
fp8 bias image directly.  All scales are powers of two -- exact in bf16.

Issue paths (three independent engine FIFOs feeding the 16 SDMA engines):
  SP   (sync HWDGE): weight tiles only.  Window 0 is split into 4 bank-
       sized pieces so the first matmul starts ~2 us earlier; window 15
       into halves so its tail overlaps the drain.
  ACT  (scalar HWDGE): the 4 x chunks up front, then one out tile per
       window (last window flushed per-bank).
  Pool (gpsimd SWDGE): 16 per-window bias DMAs, emitted up front, casting
       fp8 -> bf16 inline.  A separate descriptor ring, so bias can never
       queue behind x or outs.

Per-core compute (unchanged from v8): out.T[b, l, o] = sum_c x[c, m, b] *
W[o, c, l, m-l] per x column m.  The x column is the PE stationary operand
(K=128 c, M=32 b); the weights are the moving operand, host-packed in
exact consumption order (contiguous (l', o) blocks per (bank, x column))
because fp8 moving operands only stream at full rate when the innermost
dim is stride-1.  One PSUM bank takes 6 weight matmuls (x columns
m = 4j..4j+5 clipped to the bank); the first carries start=True.  The DVE
drains each bank with scalar_tensor_tensor (st = ps*1 + bias) fused into
the fp32->bf16 staging copy.
"""

import numpy as np
import ml_dtypes

BF16 = ml_dtypes.bfloat16
F8NP = ml_dtypes.float8_e3m4

import concourse.bass as bass
import concourse.mybir as mybir
import concourse.tile as tile
from concourse.vector_clock import ScopedClock, VectorClock
from concourse.bass_utils import run_bass_kernel_spmd

# ---------------------------------------------------------------------------
# Environment patches
# ---------------------------------------------------------------------------

# The walrus build in this image rejects instructions with >1 sem wait; the
# Tile tail drain carries one wait per logical processor.  Split them into
# single-wait nops on SP before the drain.
def _patched_drain_and_barrier(self, tick_clock, wait_clock):
    gc = tick_clock.global_clock
    n = len(gc)
    for proc in range(n):
        t = gc[proc]
        if t <= 0:
            continue
        single = VectorClock([0] * n)
        single.require_at_least(proc, t)
        inst = self.nc.sync.nop(hint="tail_drain_wait")
        wait_clock.add_sem_waits(inst.ins, ScopedClock({None: single}))
    self.nc.sync.drain()
    self.nc.all_engine_barrier()
    assert self.sems is not None
    popped = self.nc._tile_sem_poison_stack.pop()
    assert popped is self._sem_poison
    # Clear sems WITHOUT the trailing all-engine barrier: the clear runs on
    # one engine after the barrier above, and nothing after it reads sems.
    self.nc.clear_and_free_semaphores(list(self.sems.allocated().values()))


if not getattr(tile.TileContext, "_drain_patch_applied", False):
    tile.TileContext._drain_and_barrier = _patched_drain_and_barrier
    tile.TileContext._drain_patch_applied = True


def _split_multi_waits(nc: bass.Bass) -> int:
    """Hoist all but the last wait of any multi-wait instruction onto
    single-wait nops inserted just before it in its engine's program order
    (the hardware takes one sem wait per instruction; this walrus build
    rejects multi-wait instructions instead of splitting them)."""
    n_split = 0
    for f in nc.m.functions:
        for bb in f.blocks:
            insts = list(bb.instructions)
            out = []
            for inst in insts:
                si = inst.sync_info
                if si is not None and len(si.on_wait) > 1:
                    waits = list(si.on_wait)
                    for w in waits[:-1]:
                        nop = mybir.InstNoOp(
                            name=nc.get_next_instruction_name(),
                            engine=inst.engine,
                            ins=[],
                            outs=[],
                            sync_info=mybir.SyncInfo(on_wait=[w], on_update=[]),
                        )
                        out.append(nop)
                    si.on_wait = [waits[-1]]
                    n_split += 1
                out.append(inst)
            bb.instructions = out
    return n_split

# ---------------------------------------------------------------------------
# Problem constants (hardcoded from the module spec)
# ---------------------------------------------------------------------------
N_CORES = 8
B = 32
CIN = 128
COUT = 128
L = 2048
KS = 3
W_FULL = 2050

LSH = L // N_CORES          # 256 output positions per core
WW = LSH + KS - 1           # 258-wide x window per core

LT = 16                     # l positions per weight tile / staging window
NWIN = LSH // LT            # 16 windows per core
BANKL = 4                   # l positions per PSUM bank (4*128 = 512 fp32)
NBANK = LT // BANKL         # 4 banks per window
WFREE = COUT * LT * KS      # weight tile free size (6144 fp8 = 6 KB/part)
BANKF = COUT * BANKL * KS   # weight bytes per bank per partition (1536)

F32 = mybir.dt.float32
F16 = mybir.dt.bfloat16
F8 = mybir.dt.float8e3         # e3m4: 4 mantissa bits, max 15.5
WSCALE = 2.0 ** -5             # weight pre-scale so w/WSCALE fits e3m4
XSCALE = 2.0 ** -1             # x pre-scale: PSUM = (w/WSCALE)*(x*XSCALE)
                               #            = 16*w*x; bias ships as fp8(16*b)
OSCALE = WSCALE / XSCALE       # host multiplies gathered out by this (2^-4)

# per-bank x-column blocks: d = mw - lw0 in 0..5, with nl(d) l' rows each
# (nl = 1,2,3,3,2,1); DOFF[d] = block offset within the bank, in COUT units
DOFF = [0, 1, 3, 6, 9, 11]

# x chunk boundaries on the ACT queue: window lc needs cols < 16*lc + 18
XCHUNKS = [(0, LT + 2), (LT + 2, 5 * LT + 2), (5 * LT + 2, 9 * LT + 2),
           (9 * LT + 2, WW)]


def _weight_perm() -> np.ndarray:
    """Flat destination position (within a window's 6144-element image) for
    each source element ordered (l', k, o)."""
    pos = np.empty((LT, KS, COUT), dtype=np.int64)
    o = np.arange(COUT)
    for lp in range(LT):
        jb = lp // BANKL
        for k in range(KS):
            mw = lp + k
            d = mw - jb * BANKL
            lo = max(jb * BANKL, mw - (KS - 1))
            base = jb * BANKF + DOFF[d] * COUT + (lp - lo) * COUT
            pos[lp, k] = base + o
    return pos.reshape(-1)


_WPERM = _weight_perm()


def _build_nc(split: bool = True) -> bass.Bass:
    nc = bass.Bass()

    x_d = nc.declare_dram_parameter("xT", [CIN, WW, B], F16, isOutput=False)
    wt_d = nc.declare_dram_parameter("wt", [NWIN, CIN, WFREE], F8,
                                     isOutput=False)
    # bias pre-replicated by the host across the 32 batch partitions (DVE
    # lanes cannot read across partitions) and pre-scaled by 16 so it ships
    # as fp8; the SWDGE DMA expands it to bf16 in SBUF.
    br_d = nc.declare_dram_parameter("biasR", [B, LSH, COUT], F8,
                                     isOutput=False)
    # (b, l, o) layout: staging DMAs out as contiguous runs; the host
    # transposes back after gather.
    out_d = nc.declare_dram_parameter("out", [B, LSH, COUT], F16, isOutput=True)

    with tile.TileContext(nc) as tc:
        with (
            tc.tile_pool(name="xp", bufs=1) as xp,
            tc.tile_pool(name="btp", bufs=1) as btp,
            tc.tile_pool(name="wp", bufs=12) as wp,
            tc.tile_pool(name="sp", bufs=6) as sp,
            tc.tile_pool(name="pp", bufs=8, space="PSUM") as pp,
        ):
            # Persistent x in (c, w, b) layout: the stationary operand for
            # column m is x_sb[:, m, :] (K=128 c, M=32 b).  Host pre-
            # transposed, so both DMA sides are fully contiguous.  The first
            # chunk covers only window 0 so the first matmul starts ASAP.
            x_sb = xp.tile([CIN, WW, B], F16)
            b_all = btp.tile([B, LSH, COUT], F8)

            def bias_load(w):
                nc.scalar.dma_start(b_all[:, w * LT:(w + 1) * LT, :],
                                    br_d[:, w * LT:(w + 1) * LT, :])

            # ACT queue prefix, in need order: window-0 x, windows 1-4 x,
            # first two bias windows; later x chunks + bias ride between
            # the out tiles inside the loop.
            nc.scalar.dma_start(x_sb[:, 0:XCHUNKS[0][1], :],
                                x_d[:, 0:XCHUNKS[0][1], :])
            a, b_ = XCHUNKS[1]
            nc.scalar.dma_start(x_sb[:, a:b_, :], x_d[:, a:b_, :])
            bias_load(0)
            bias_load(1)

            # PE warm-up: the HAM clock gate holds the PE at 1.2 GHz until
            # it has seen ~3.4 us of sustained activity.  The first weight
            # tile lands ~2.5 us after the preamble barrier; matmuls on a
            # memset scratch tile (no DMA dependency, so they start
            # immediately) fill that window so the real windows run at
            # 2.4 GHz from the start.  The scratch PSUM bank is never read.
            scr = xp.tile([CIN, 512], F16, name="warm_src")
            nc.vector.memset(scr[:], 0.0)
            warm = pp.tile([B, BANKL, COUT], F32, tag="ps", name="warm")
            for _ in range(8):
                nc.tensor.matmul(
                    warm[:], scr[:, 0:B], scr[:],
                    start=True, stop=True, skip_group_check=True,
                )

            for lc in range(NWIN):
                if lc + 2 < NWIN:
                    bias_load(lc + 2)
                if lc == 0:
                    a, b_ = XCHUNKS[2]
                    nc.scalar.dma_start(x_sb[:, a:b_, :], x_d[:, a:b_, :])
                if lc == 2:
                    a, b_ = XCHUNKS[3]
                    nc.scalar.dma_start(x_sb[:, a:b_, :], x_d[:, a:b_, :])
                # weight tile, host-packed in matmul consumption order: for
                # each bank jb and x column d, a contiguous (l', o) block.
                # fp8 moving operands stream at full rate ONLY when the
                # innermost dim is stride-1.  One big DMA per window (small
                # DMAs tank the stream rate); window 0 is split per-bank so
                # bank 0's matmuls start ~2us earlier, window 15 in halves
                # so its compute overlaps the stream tail.
                w_t = wp.tile([CIN, WFREE], F8, tag="w", name="w_t")
                if lc == 0:
                    for jb in range(NBANK):
                        nc.sync.dma_start(
                            w_t[:, jb * BANKF:(jb + 1) * BANKF],
                            wt_d[lc, :, jb * BANKF:(jb + 1) * BANKF])
                elif lc == NWIN - 1:
                    half = WFREE // 2
                    nc.sync.dma_start(w_t[:, 0:half], wt_d[lc, :, 0:half])
                    nc.sync.dma_start(w_t[:, half:WFREE],
                                      wt_d[lc, :, half:WFREE])
                else:
                    nc.sync.dma_start(w_t[:], wt_d[lc])

                st = sp.tile([B, LT, COUT], F16, tag="st", name=f"st_{lc}")

                for jb in range(NBANK):
                    ps = pp.tile([B, BANKL, COUT], F32, tag="ps", name="ps")
                    lw0 = jb * BANKL              # window-local l of bank start

                    # six weight matmuls: x columns m = bank start .. +5;
                    # each reads one fully contiguous nl*COUT-element block.
                    # d=0 carries start=True: clears the bank's has_written
                    # bits, so each element's first matmul overwrites and
                    # later ones accumulate -- no separate init matmul.
                    for d in range(BANKL + KS - 1):
                        mw = lw0 + d                  # window-local x column
                        m = lc * LT + mw              # shard-local x column
                        lo = max(lw0, mw - (KS - 1))  # window-local l' range
                        hi = min(lw0 + BANKL - 1, mw)
                        nl = hi - lo + 1
                        rhs = bass.AP(
                            w_t[:].tensor,
                            jb * BANKF + DOFF[d] * COUT,
                            [[WFREE, CIN], [1, nl * COUT]],
                        )
                        nc.tensor.matmul(
                            ps[:, lo - lw0:hi - lw0 + 1, :],
                            x_sb[:, m, :],
                            rhs,
                            start=(d == 0),
                            stop=(d == BANKL + KS - 2),
                            skip_group_check=True,
                        )

                    # staging = PSUM + bias, fused into the DVE copy:
                    # st = (ps * 1.0) + bias  (fp32 PSUM -> bf16 SBUF)
                    nc.vector.scalar_tensor_tensor(
                        st[:, lw0:lw0 + BANKL, :],
                        ps[:],
                        1.0,
                        b_all[:, lc * LT + lw0:lc * LT + lw0 + BANKL, :],
                        op0=mybir.AluOpType.mult,
                        op1=mybir.AluOpType.add,
                    )

                if lc < NWIN - 1:
                    nc.scalar.dma_start(out_d[:, lc * LT:(lc + 1) * LT, :],
                                        st[:])
                else:
                    # last window: per-bank flushes so the kernel tail is one
                    # small transfer instead of a whole-window one
                    for jb in range(NBANK):
                        l0 = lc * LT + jb * BANKL
                        nc.scalar.dma_start(
                            out_d[:, l0:l0 + BANKL, :],
                            st[:, jb * BANKL:(jb + 1) * BANKL, :])

    if split:
        _split_multi_waits(nc)
    return nc


_NC_CACHE = None


def _get_nc() -> bass.Bass:
    global _NC_CACHE
    if _NC_CACHE is None:
        _NC_CACHE = _build_nc()
    return _NC_CACHE


def _tile_weights(w_shard: np.ndarray) -> np.ndarray:
    """(COUT, CIN, LSH, KS) -> (NWIN, CIN, WFREE) per-window SBUF tile
    images in matmul consumption order: contiguous (l', o) blocks per
    (bank, x-column), so every matmul rhs is one stride-1 run."""
    w = w_shard.transpose(1, 2, 3, 0)                  # (CIN, LSH, KS, COUT)
    w = w.reshape(CIN, NWIN, LT * KS * COUT)
    w = np.ascontiguousarray(w.transpose(1, 0, 2))     # (NWIN, CIN, LT*KS*COUT)
    out = np.empty_like(w)
    out[:, :, _WPERM] = w
    return out


def shard_inputs(x, weight, bias):
    x = (np.asarray(x, dtype=np.float32) * XSCALE).astype(BF16)
    weight = (np.asarray(weight, dtype=np.float32) * (1.0 / WSCALE)).astype(F8NP)
    bias = (np.asarray(bias, dtype=np.float32) * (1.0 / OSCALE)).astype(F8NP)
    xT = x.transpose(1, 2, 0)                          # (CIN, W_FULL, B)
    in_maps = []
    for i in range(N_CORES):
        l0 = i * LSH
        in_maps.append({
            "xT": np.ascontiguousarray(xT[:, l0:l0 + WW, :]),
            "wt": _tile_weights(weight[:, :, l0:l0 + LSH, :]),
            "biasR": np.ascontiguousarray(
                np.broadcast_to(bias[:, l0:l0 + LSH].T[None, :, :],
                                (B, LSH, COUT))),
        })
    return in_maps


def gather_output(results):
    out = np.empty((B, COUT, L), dtype=np.float32)
    for i in range(N_CORES):
        out[:, :, i * LSH:(i + 1) * LSH] = (
            results[i]["out"].astype(np.float32).transpose(0, 2, 1) * OSCALE)
    return out


def kernel(x, weight, bias):
    nc = _get_nc()
    in_maps = shard_inputs(x, weight, bias)
    res = run_bass_kernel_spmd(nc, in_maps, core_ids=list(range(N_CORES)),
                               trace=False)
    return gather_output(res.results)
